# revision 3
# baseline (speedup 1.0000x reference)
"""Trainium2 Bass kernel for nn_EquivariantUpSampling_72773925864032.

Op (derived from the reference, verified numerically):
  inputs  x: (8, 128, 32, 32) f32,  p: (8, 3) int64 with entries in {0, 1}
  output  (8, 256, 64, 64) f32, zeros except, per sample i with
  (ph, pw, r) = p[i]:
      out[i, 2c + r, 2a + ph, 2b + pw] = rot_r(x[i, c])[a, b]
  where rot_0 = identity and rot_1[a, b] = x[b, (32 - a) % 32]
  (only r in {0, 1} is reachable since p = randint(0, 2)).

Strategy: pure data parallel, one sample per NeuronCore (8 cores), all
data in bf16 (the correctness gate is rel_err < 2e-2; bf16 rounding of x
costs ~3e-3 and halves every DMA byte). Per core:

  * The entire data-dependent placement is folded into ONE host-computed
    dynamic DMA offset off = r*4096 + ph*64 + pw applied via ds() to a
    flat padded view of out: shifting each contiguous 4096-element
    channel write by off lands data exactly at (2c+r, 2a+ph, 2b+pw).
    The <=65-element tail spill into the next channel's head writes only
    zeros over zero positions (or the pad tail for the last channel).
  * SBUF tensor T4 holds the full upsampled channel (64x64, zeros
    interleaved). Even rows (data rows for the pre-shift layout) are
    produced by a uint16->uint32 tensor_copy of the selected bf16 row:
    zero-extension writes little-endian [t, 0x0000] pairs, i.e. the data
    column AND the interleaved zero column in one contiguous full-rate
    pass. Only odd rows need a memset, which fits entirely before the
    input DMA lands. The rotation select itself is one fused
    scalar_tensor_tensor per row block: tSel = rot_AP*w1 + (x*w0), with
    one-hot w0/w1 host-provided per core (SPMD: one program, 8 cores).
  * Output: 4 DMAs (channel-half x row-half) of contiguous 4 KiB
    descriptors over both HWDGE queues (small descriptors are
    packet-rate-bound at ~27 ns/descriptor/engine on trn2 — descriptor
    count, not bytes, is what matters). The 128 never-written channels
    and all remaining zeros rely on the zero-initialized output buffer
    (run_bass_kernel_spmd / bass2jax contract).
  * No gpsimd: Pool-engine work during the input transfer contends with
    SDMA engine 15 via the SWDGE descriptor-ring SBUF ports (measured
    ~2.7 us input stragglers).

Measured (8-core SPMD, NTFF trace): ~18.6-19.0 us mean / 18.9-20.7 us max
per-core exec vs 24.4/28.1 us for the f32 one-hot-quad baseline. The
residual is dominated by fixed NEFF protocol overhead (startup barrier,
preamble loads, DMA completion receipts, semaphore-clear epilogue).
"""

import numpy as np

B, C, H, W = 8, 128, 32, 32
OC, OH, OW = 2 * C, 2 * H, 2 * W
N_CORES = 8
NW = 16  # header cols (bf16): [off:int32][pad][pad][w1:f32][w0:f32][pad...]
XCOLS = NW + H * W  # 1040 bf16 per partition
OUT_ELEMS = OC * OH * OW  # 1048576
OUT_PAD = OUT_ELEMS + 4608  # ds window off+OUT_ELEMS must fit; off <= 4161

_compiled = {}


def _build_bass():
    from contextlib import ExitStack

    import concourse.bass as bass
    import concourse.mybir as mybir

    bf16 = mybir.dt.bfloat16
    orig_aeb = bass.Bass.all_engine_barrier
    bass.Bass.all_engine_barrier = lambda self, **kw: None
    try:
        nc = bass.Bass(enable_partition_id=False)
    finally:
        bass.Bass.all_engine_barrier = orig_aeb

    SP = mybir.EngineType.SP
    ACT = mybir.EngineType.Activation

    xw = nc.dram_tensor("xw", (C, XCOLS), bf16, kind="ExternalInput")
    out = nc.dram_tensor("out", (1, OUT_PAD), bf16, kind="ExternalOutput")

    ctx = ExitStack()
    with ctx:
        xin = ctx.enter_context(nc.sbuf_tensor([C, XCOLS], bf16))
        tA = ctx.enter_context(nc.sbuf_tensor([C, H * W], bf16))
        tSel = ctx.enter_context(nc.sbuf_tensor([C, H * W], bf16))
        T4 = ctx.enter_context(nc.sbuf_tensor([C, OH * OW], bf16))  # 4096

        s_a = nc.alloc_semaphore("s_a")  # input half A (header + x rows 0:16)
        s_b = nc.alloc_semaphore("s_b")  # input half B (x rows 16:32)
        s_v = nc.alloc_semaphore("s_v")  # DVE progress (in-order completions)
        s_out = nc.alloc_semaphore("s_out")

        x3 = xin[:, NW : NW + 1024].rearrange("p (a b) -> p a b", b=W)
        tA3 = tA[:].rearrange("p (a b) -> p a b", b=W)
        tS3 = tSel[:].rearrange("p (a b) -> p a b", b=W)
        tSu3 = tSel[:].bitcast(mybir.dt.uint16).rearrange("p (a b) -> p a b", b=W)
        T4r = T4[:].rearrange("p (row col) -> p row col", col=OW)
        T4w3 = T4[:].bitcast(mybir.dt.uint32).rearrange(
            "p (row w) -> p row w", w=OW // 2
        )
        w1ap = xin[:, 6:8].bitcast(mybir.dt.float32)
        w0ap = xin[:, 8:10].bitcast(mybir.dt.float32)
        offw = xin[:, 0:2].bitcast(mybir.dt.int32)

        oflat = out[0]

        cut = NW + (H // 2) * W  # 528
        nc.sync.dma_start(xin[:, 0:cut], xw[:, 0:cut]).then_inc(s_a, 16)
        nc.scalar.dma_start(xin[:, cut:XCOLS], xw[:, cut:XCOLS]).then_inc(s_b, 16)

        nc.sync.wait_ge(s_a, 16)
        nc.scalar.wait_ge(s_a, 16)
        off = nc.values_load(
            offw[0:1, 0:1],
            engines=[SP, ACT],
            min_val=0,
            max_val=OH * OW + OW + 1,  # 4161
            skip_runtime_bounds_check=True,
        )
        # channel 2c+r occupies elems off + c*8192 .. +4096 of the flat view
        dst = oflat[bass.ds(off, OUT_ELEMS)].rearrange(
            "(c j) -> c j", c=C, j=2 * OH * OW
        )

        # ---- DVE: memset odd rows only (even rows fully written by the
        # zext copies), tA, select into tSel, zext-scatter into T4 ----
        # s_v in-order completions: memset=1, mul0=2, mul1=3, stt0a=4,
        # stt0b=5, conv0=6, stt1=7, conv1=8.
        nc.vector.memset(T4r[:, 1:OH:2, :], 0.0).then_inc(s_v, 1)
        nc.vector.wait_ge(s_a, 16)
        nc.vector.tensor_scalar_mul(
            tA[:, 0 : 512], xin[:, NW : NW + 512], w0ap
        ).then_inc(s_v, 1)  # packed bf16 -> 2x
        nc.vector.wait_ge(s_b, 16)
        nc.vector.tensor_scalar_mul(
            tA[:, 512:1024], xin[:, NW + 512 : NW + 1024], w0ap
        ).then_inc(s_v, 1)
        # tSel[a,b] = rot1(x)[a,b]*w1 + tA[a,b]  (rot1[a,b] = x[b,(32-a)%32]);
        # then T4 even rows via zext: u32 word [t,0x0000] = data + zero col.
        mult, add = mybir.AluOpType.mult, mybir.AluOpType.add
        nc.vector.wait_ge(s_v, 2)
        nc.vector.scalar_tensor_tensor(
            tS3[:, 0:1, :],
            x3[:, :, 0:1].transpose([0, 2, 1]),
            w1ap,
            tA3[:, 0:1, :],
            mult,
            add,
        ).then_inc(s_v, 1)
        nc.vector.scalar_tensor_tensor(
            tS3[:, 1:16, :],
            x3[:, :, 31:16:-1].transpose([0, 2, 1]),
            w1ap,
            tA3[:, 1:16, :],
            mult,
            add,
        ).then_inc(s_v, 1)
        nc.vector.wait_ge(s_v, 5)
        nc.vector.tensor_copy(T4w3[:, 0:OH // 2 : 2, :], tSu3[:, 0:16, :]).then_inc(
            s_v, 1
        )
        nc.vector.scalar_tensor_tensor(
            tS3[:, 16:32, :],
            x3[:, :, 16:0:-1].transpose([0, 2, 1]),
            w1ap,
            tA3[:, 16:32, :],
            mult,
            add,
        ).then_inc(s_v, 1)
        nc.vector.wait_ge(s_v, 7)
        nc.vector.tensor_copy(
            T4w3[:, OH // 2 : OH : 2, :], tSu3[:, 16:32, :]
        ).then_inc(s_v, 1)

        # ---- output: 4 DMAs (channel-half x row-half), 4 KiB descs ----
        nc.sync.wait_ge(s_v, 6)
        nc.sync.dma_start(dst[0:64, 0:2048], T4[0:64, 0:2048]).then_inc(s_out, 16)
        nc.scalar.wait_ge(s_v, 6)
        nc.scalar.dma_start(dst[64:128, 0:2048], T4[64:128, 0:2048]).then_inc(
            s_out, 16
        )
        nc.sync.wait_ge(s_v, 8)
        nc.sync.dma_start(dst[0:64, 2048:4096], T4[0:64, 2048:4096]).then_inc(
            s_out, 16
        )
        nc.scalar.wait_ge(s_v, 8)
        nc.scalar.dma_start(dst[64:128, 2048:4096], T4[64:128, 2048:4096]).then_inc(
            s_out, 16
        )
        nc.sync.wait_ge(s_out, 64)
    return nc


def _get_bass():
    if "nc" not in _compiled:
        _compiled["nc"] = _build_bass()
    return _compiled["nc"]


def _make_in_maps(x, p):
    import ml_dtypes

    bf = ml_dtypes.bfloat16
    x = np.asarray(x, dtype=np.float32)
    p = np.asarray(p)
    in_maps = []
    for i in range(B):
        ph, pw, r = int(p[i, 0]), int(p[i, 1]), int(p[i, 2])
        assert r in (0, 1) and ph in (0, 1) and pw in (0, 1)
        buf = np.zeros((C, XCOLS), bf)
        hdr32 = buf.view(np.int32)
        hdrf = buf.view(np.float32)
        hdr32[:, 0] = r * (OH * OW) + ph * OW + pw
        hdrf[:, 3] = 1.0 if r == 1 else 0.0  # w1 (rot branch)
        hdrf[:, 4] = 1.0 if r == 0 else 0.0  # w0 (identity branch)
        buf[:, NW:] = x[i].reshape(C, H * W).astype(bf)
        in_maps.append({"xw": buf})
    return in_maps


def run(x, p, **spmd_kwargs):
    """Run the Bass kernel on 8 cores; returns (output, BassKernelResults)."""
    from concourse.bass_utils import run_bass_kernel_spmd

    nc = _get_bass()
    in_maps = _make_in_maps(x, p)
    res = run_bass_kernel_spmd(
        nc, in_maps, core_ids=list(range(N_CORES)), **spmd_kwargs
    )
    out = np.stack(
        [
            np.asarray(res.results[i]["out"])
            .reshape(-1)[:OUT_ELEMS]
            .astype(np.float32)
            .reshape(OC, OH, OW)
            for i in range(B)
        ],
        axis=0,
    )
    return out, res


def kernel(x, p):
    out, _ = run(x, p)
    return out


# revision 4
# speedup vs baseline: 1.0965x; 1.0965x over previous
"""Trainium2 Bass kernel for nn_EquivariantUpSampling_72773925864032.

Op (derived from the reference, verified numerically):
  inputs  x: (8, 128, 32, 32) f32,  p: (8, 3) int64 with entries in {0, 1}
  output  (8, 256, 64, 64) f32, zeros except, per sample i with
  (ph, pw, r) = p[i]:
      out[i, 2c + r, 2a + ph, 2b + pw] = rot_r(x[i, c])[a, b]
  where rot_0 = identity and rot_1[a, b] = x[b, (32 - a) % 32]
  (only r in {0, 1} is reachable since p = randint(0, 2)).

Strategy: pure data parallel, one sample per NeuronCore (8 cores), all
data in bf16 (the correctness gate is rel_err < 2e-2; bf16 rounding of x
costs ~3e-3 and halves every DMA byte). Per core:

  * The entire data-dependent placement is folded into ONE host-computed
    dynamic DMA offset off = r*4096 + ph*64 + pw applied via ds() to a
    flat padded view of out: shifting each contiguous 4096-element
    channel write by off lands data exactly at (2c+r, 2a+ph, 2b+pw).
    The <=65-element tail spill into the next channel's head writes only
    zeros over zero positions (or the pad tail for the last channel).
  * SBUF tensor T4 holds the full upsampled channel (64x64, zeros
    interleaved). Even rows (data rows for the pre-shift layout) are
    produced by a uint16->uint32 tensor_copy of the selected bf16 row:
    zero-extension writes little-endian [t, 0x0000] pairs, i.e. the data
    column AND the interleaved zero column in one contiguous full-rate
    pass. Only odd rows need a memset, which fits entirely before the
    input DMA lands. The rotation select itself is one fused
    scalar_tensor_tensor per row block: tSel = rot_AP*w1 + (x*w0), with
    one-hot w0/w1 host-provided per core (SPMD: one program, 8 cores).
  * Output: 4 DMAs (channel-half x row-half) of contiguous 4 KiB
    descriptors over both HWDGE queues (small descriptors are
    packet-rate-bound at ~27 ns/descriptor/engine on trn2 — descriptor
    count, not bytes, is what matters). The 128 never-written channels
    and all remaining zeros rely on the zero-initialized output buffer
    (run_bass_kernel_spmd / bass2jax contract).
  * No gpsimd: Pool-engine work during the input transfer contends with
    SDMA engine 15 via the SWDGE descriptor-ring SBUF ports (measured
    ~2.7 us input stragglers).

Measured (8-core SPMD, NTFF trace): ~18.6-19.0 us mean / 18.9-20.7 us max
per-core exec vs 24.4/28.1 us for the f32 one-hot-quad baseline. The
residual is dominated by fixed NEFF protocol overhead (startup barrier,
preamble loads, DMA completion receipts, semaphore-clear epilogue).
"""

import numpy as np

B, C, H, W = 8, 128, 32, 32
OC, OH, OW = 2 * C, 2 * H, 2 * W
N_CORES = 8
NW = 16  # header cols (bf16): [off:int32][pad][pad][w1:f32][w0:f32][pad...]
XCOLS = NW + H * W  # 1040 bf16 per partition
OUT_ELEMS = OC * OH * OW  # 1048576
OUT_PAD = OUT_ELEMS + 4608  # ds window off+OUT_ELEMS must fit; off <= 4161

_compiled = {}


def _build_bass():
    from contextlib import ExitStack

    import concourse.bass as bass
    import concourse.mybir as mybir

    bf16 = mybir.dt.bfloat16
    orig_aeb = bass.Bass.all_engine_barrier
    bass.Bass.all_engine_barrier = lambda self, **kw: None
    try:
        nc = bass.Bass(enable_partition_id=False)
    finally:
        bass.Bass.all_engine_barrier = orig_aeb

    SP = mybir.EngineType.SP
    ACT = mybir.EngineType.Activation

    xw = nc.dram_tensor("xw", (C, XCOLS), bf16, kind="ExternalInput")
    out = nc.dram_tensor("out", (1, OUT_PAD), bf16, kind="ExternalOutput")

    ctx = ExitStack()
    with ctx:
        xin = ctx.enter_context(nc.sbuf_tensor([C, XCOLS], bf16))
        tA = ctx.enter_context(nc.sbuf_tensor([C, H * W], bf16))
        tSel = ctx.enter_context(nc.sbuf_tensor([C, H * W], bf16))
        T4 = ctx.enter_context(nc.sbuf_tensor([C, OH * OW], bf16))  # 4096

        s_a = nc.alloc_semaphore("s_a")  # input half A (header + x rows 0:16)
        s_b = nc.alloc_semaphore("s_b")  # input half B (x rows 16:32)
        s_v = nc.alloc_semaphore("s_v")  # DVE progress (in-order completions)
        s_out = nc.alloc_semaphore("s_out")

        x3 = xin[:, NW : NW + 1024].rearrange("p (a b) -> p a b", b=W)
        tA3 = tA[:].rearrange("p (a b) -> p a b", b=W)
        tS3 = tSel[:].rearrange("p (a b) -> p a b", b=W)
        tSu3 = tSel[:].bitcast(mybir.dt.uint16).rearrange("p (a b) -> p a b", b=W)
        T4r = T4[:].rearrange("p (row col) -> p row col", col=OW)
        T4w3 = T4[:].bitcast(mybir.dt.uint32).rearrange(
            "p (row w) -> p row w", w=OW // 2
        )
        w1ap = xin[:, 6:8].bitcast(mybir.dt.float32)
        w0ap = xin[:, 8:10].bitcast(mybir.dt.float32)
        offw = xin[:, 0:2].bitcast(mybir.dt.int32)

        oflat = out[0]

        cut = NW + (H // 2) * W  # 528
        nc.sync.dma_start(xin[:, 0:cut], xw[:, 0:cut]).then_inc(s_a, 16)
        nc.scalar.dma_start(xin[:, cut:XCOLS], xw[:, cut:XCOLS]).then_inc(s_b, 16)

        nc.sync.wait_ge(s_a, 16)
        nc.scalar.wait_ge(s_a, 16)
        off = nc.values_load(
            offw[0:1, 0:1],
            engines=[SP, ACT],
            min_val=0,
            max_val=OH * OW + OW + 1,  # 4161
            skip_runtime_bounds_check=True,
        )
        # channel 2c+r occupies elems off + c*8192 .. +4096 of the flat view
        dst = oflat[bass.ds(off, OUT_ELEMS)].rearrange(
            "(c j) -> c j", c=C, j=2 * OH * OW
        )

        # ---- DVE: memset odd rows only (even rows fully written by the
        # zext copies), tA, select into tSel, zext-scatter into T4 ----
        # s_v in-order completions: memset=1, mul0=2, mul1=3, stt0a=4,
        # stt0b=5, conv0=6, stt1=7, conv1=8.
        # odd rows zeroed via the u32 view: same bytes, half the elements
        nc.vector.memset(T4w3[:, 1:OH:2, :], 0).then_inc(s_v, 1)
        nc.vector.wait_ge(s_a, 16)
        nc.vector.tensor_scalar_mul(
            tA[:, 0 : 512], xin[:, NW : NW + 512], w0ap
        ).then_inc(s_v, 1)  # packed bf16 -> 2x
        nc.vector.wait_ge(s_b, 16)
        nc.vector.tensor_scalar_mul(
            tA[:, 512:1024], xin[:, NW + 512 : NW + 1024], w0ap
        ).then_inc(s_v, 1)
        # tSel[a,b] = rot1(x)[a,b]*w1 + tA[a,b]  (rot1[a,b] = x[b,(32-a)%32]);
        # then T4 even rows via zext: u32 word [t,0x0000] = data + zero col.
        mult, add = mybir.AluOpType.mult, mybir.AluOpType.add
        nc.vector.wait_ge(s_v, 2)
        nc.vector.scalar_tensor_tensor(
            tS3[:, 0:1, :],
            x3[:, :, 0:1].transpose([0, 2, 1]),
            w1ap,
            tA3[:, 0:1, :],
            mult,
            add,
        ).then_inc(s_v, 1)
        nc.vector.scalar_tensor_tensor(
            tS3[:, 1:16, :],
            x3[:, :, 31:16:-1].transpose([0, 2, 1]),
            w1ap,
            tA3[:, 1:16, :],
            mult,
            add,
        ).then_inc(s_v, 1)
        nc.vector.wait_ge(s_v, 5)
        nc.vector.tensor_copy(T4w3[:, 0:OH // 2 : 2, :], tSu3[:, 0:16, :]).then_inc(
            s_v, 1
        )
        nc.vector.scalar_tensor_tensor(
            tS3[:, 16:32, :],
            x3[:, :, 16:0:-1].transpose([0, 2, 1]),
            w1ap,
            tA3[:, 16:32, :],
            mult,
            add,
        ).then_inc(s_v, 1)
        nc.vector.wait_ge(s_v, 7)
        nc.vector.tensor_copy(
            T4w3[:, OH // 2 : OH : 2, :], tSu3[:, 16:32, :]
        ).then_inc(s_v, 1)

        # ---- output: 4 DMAs (channel-half x row-half), 4 KiB descs ----
        nc.sync.wait_ge(s_v, 6)
        nc.sync.dma_start(dst[0:64, 0:2048], T4[0:64, 0:2048]).then_inc(s_out, 16)
        nc.scalar.wait_ge(s_v, 6)
        nc.scalar.dma_start(dst[64:128, 0:2048], T4[64:128, 0:2048]).then_inc(
            s_out, 16
        )
        nc.sync.wait_ge(s_v, 8)
        nc.sync.dma_start(dst[0:64, 2048:4096], T4[0:64, 2048:4096]).then_inc(
            s_out, 16
        )
        nc.scalar.wait_ge(s_v, 8)
        nc.scalar.dma_start(dst[64:128, 2048:4096], T4[64:128, 2048:4096]).then_inc(
            s_out, 16
        )
        nc.sync.wait_ge(s_out, 64)
    return nc


def _get_bass():
    if "nc" not in _compiled:
        _compiled["nc"] = _build_bass()
    return _compiled["nc"]


def _make_in_maps(x, p):
    import ml_dtypes

    bf = ml_dtypes.bfloat16
    x = np.asarray(x, dtype=np.float32)
    p = np.asarray(p)
    in_maps = []
    for i in range(B):
        ph, pw, r = int(p[i, 0]), int(p[i, 1]), int(p[i, 2])
        assert r in (0, 1) and ph in (0, 1) and pw in (0, 1)
        buf = np.zeros((C, XCOLS), bf)
        hdr32 = buf.view(np.int32)
        hdrf = buf.view(np.float32)
        hdr32[:, 0] = r * (OH * OW) + ph * OW + pw
        hdrf[:, 3] = 1.0 if r == 1 else 0.0  # w1 (rot branch)
        hdrf[:, 4] = 1.0 if r == 0 else 0.0  # w0 (identity branch)
        buf[:, NW:] = x[i].reshape(C, H * W).astype(bf)
        in_maps.append({"xw": buf})
    return in_maps


def run(x, p, **spmd_kwargs):
    """Run the Bass kernel on 8 cores; returns (output, BassKernelResults)."""
    from concourse.bass_utils import run_bass_kernel_spmd

    nc = _get_bass()
    in_maps = _make_in_maps(x, p)
    res = run_bass_kernel_spmd(
        nc, in_maps, core_ids=list(range(N_CORES)), **spmd_kwargs
    )
    out = np.stack(
        [
            np.asarray(res.results[i]["out"])
            .reshape(-1)[:OUT_ELEMS]
            .astype(np.float32)
            .reshape(OC, OH, OW)
            for i in range(B)
        ],
        axis=0,
    )
    return out, res


def kernel(x, p):
    out, _ = run(x, p)
    return out
